# revision 9
# baseline (speedup 1.0000x reference)
"""nn_Cvx_ShortestPathNet — TRN2 Bass kernel, 8-core pure data parallelism.

Math (derived from the reference's Dykstra iteration):
    G = A' pinv(AA') A   (orthogonal projector, 760x760), c = b' pinv(AA') A
    w = MLP(d); t_1 = -w
    for k = 1..K:  corr_k = t_k @ G - c ;  t_{k+1} = max(-w, corr_k)
    y = max(-w - corr_K, 0)
(The invariant t2 + p == -w collapses Dykstra's three-sequence state to a
single iterate.)

Schedule (v2):
  * c is folded into G: pad row 767 of G holds -c, G[767,767] = 1, and
    negw[767] is forced to 1, so t[767] == 1 is an invariant and the PE
    accumulates "t@G - c" directly in PSUM.  The per-iteration DVE work
    is then just 2 tensor_tensor MAX ops ([128,96] each) instead of 6
    scalar_tensor_tensor ops.
  * k-split pipelining: each iteration's 36 matmuls are emitted as
    phase A (contraction tiles k=0..2, reading tA) then phase B
    (k=3..5, reading tB).  The MAX producing the next tA only needs
    psum j=0..2, which completes 9 matmuls into phase B, so the DVE
    overlaps the PE and the PE never stalls at iteration boundaries.
  * all big operands (G, W1, W2, d) are fp16: halves the HBM traffic
    and keeps every matmul on the PE fast path.

Batch 256 is sharded 32 rows per core; G and MLP weights replicated.
"""

import json
import numpy as np

import concourse.bass as bass
import concourse.mybir as mybir
import concourse.tile as tile
from concourse.bass_utils import run_bass_kernel_spmd

F32 = mybir.dt.float32
F16 = mybir.dt.float16
AT = mybir.AluOpType
AF = mybir.ActivationFunctionType

JT = 6          # 768/128 edge-dim tiles
BL = 32         # batch rows per core
HT = 5          # 640/128 hidden tiles
K_ITERS = 100
N_CORES = 8
N2 = 760

# ---------------------------------------------------------------------------
# This container's walrus build rejects instructions carrying more than one
# sync-wait. Split any multi-wait instruction at the BIR-JSON level: insert
# same-engine NoOps before it, each carrying one of the extra waits (waits
# are sem-ge, so order is irrelevant).
_orig_to_json_bytes = bass.Bass.to_json_bytes
_ctr = [0]


def _split_waits_json(raw: bytes) -> bytes:
    j = json.loads(raw)
    changed = False
    for fn in j.get("functions", []):
        for bb in fn.get("blocks", []):
            out = []
            for inst in bb.get("instructions", []):
                si = inst.get("sync_info") or {}
                waits = si.get("on_wait") or []
                if len(waits) > 1:
                    changed = True
                    for w in waits[:-1]:
                        _ctr[0] += 1
                        out.append({
                            "debug": inst.get("debug", 0),
                            "engine": inst["engine"],
                            "ins": [], "outs": [],
                            "name": f"I-waitsplit-{_ctr[0]}",
                            "opcode": "NoOp",
                            "sync_info": {"on_wait": [w], "on_update": []},
                        })
                    si["on_wait"] = waits[-1:]
                out.append(inst)
            bb["instructions"] = out
    return json.dumps(j).encode() if changed else raw


def _patched_to_json_bytes(self, *a, **k):
    return _split_waits_json(_orig_to_json_bytes(self, *a, **k))


bass.Bass.to_json_bytes = _patched_to_json_bytes


def _build(k_iters=K_ITERS):
    nc = bass.Bass("TRN2", target_bir_lowering=False, debug=False,
                   num_devices=N_CORES)

    g_mat = nc.dram_tensor("g_mat", [128, JT * JT * 128], F16, kind="ExternalInput").ap()
    w2t = nc.dram_tensor("w2t", [128, HT * JT * 128], F16, kind="ExternalInput").ap()
    w1 = nc.dram_tensor("w1", [64, HT * 128], F16, kind="ExternalInput").ap()
    dt_in = nc.dram_tensor("dt_in", [64, BL], F16, kind="ExternalInput").ap()
    b1c = nc.dram_tensor("b1c", [128, HT], F32, kind="ExternalInput").ap()
    nb2c = nc.dram_tensor("nb2c", [128, JT], F32, kind="ExternalInput").ap()
    y_out = nc.dram_tensor("y_out", [128, JT * BL], F32, kind="ExternalOutput").ap()

    with tile.TileContext(nc) as tc:
        with (
            tc.tile_pool(name="const", bufs=1) as cpool,
            tc.tile_pool(name="state", bufs=2) as spool,
            tc.tile_pool(name="psum", bufs=2, space="PSUM") as ppool,
        ):
            # small loads first on the sync queue so the MLP starts early
            dT_sb = cpool.tile([64, BL], F16)
            nc.sync.dma_start(out=dT_sb[:], in_=dt_in[:])
            b1c_sb = cpool.tile([128, HT], F32)
            nc.sync.dma_start(out=b1c_sb[:], in_=b1c[:])
            nb2c_sb = cpool.tile([128, JT], F32)
            nc.sync.dma_start(out=nb2c_sb[:], in_=nb2c[:])
            w1_sb = cpool.tile([64, HT * 128], F16)
            nc.sync.dma_start(out=w1_sb[:], in_=w1[:])
            w2_sb = cpool.tile([128, HT * JT * 128], F16)
            nc.sync.dma_start(out=w2_sb[:], in_=w2t[:])
            # G on the SWDGE path so it overlaps the W2 load
            G_sb = cpool.tile([128, JT * JT * 128], F16)
            nc.gpsimd.dma_start(out=G_sb[:], in_=g_mat[:])

            # warm the scalar-engine activation table while DMAs run
            warm = spool.tile([128, 1], F32, tag="warm", name="warm")
            nc.scalar.activation(out=warm[:], in_=b1c_sb[:, 0:1],
                                 func=AF.Identity, scale=1.0)

            # MLP: h = leaky_relu(d@W1 + b1), negw = -(h@W2 + b2)
            h_sb = cpool.tile([128, HT * BL], F16)
            for m in range(HT):
                ph = ppool.tile([128, BL], F32, tag="mlp")
                nc.tensor.matmul(out=ph[:], lhsT=w1_sb[:, m * 128:(m + 1) * 128],
                                 rhs=dT_sb[:], start=True, stop=True)
                pre = spool.tile([128, BL], F32, tag="pre", name=f"pre{m}")
                nc.scalar.activation(out=pre[:], in_=ph[:], func=AF.Identity,
                                     bias=b1c_sb[:, m:m + 1], scale=1.0)
                # leaky relu: max(x, 0.1x) on DVE (Lrelu alpha is hardcoded
                # to 0.01 in this compiler build)
                nc.vector.scalar_tensor_tensor(
                    out=h_sb[:, m * BL:(m + 1) * BL], in0=pre[:],
                    scalar=0.1, in1=pre[:], op0=AT.mult, op1=AT.max)

            # negw: 6 activations into j-major [128, 6, 32], then one DVE
            # rearrange copy per half-batch problem into [128, 6*16] layout
            negw32 = cpool.tile([128, JT, BL], F32)
            for j in range(JT):
                pw = ppool.tile([128, BL], F32, tag="mlp")
                for k2 in range(HT):
                    nc.tensor.matmul(
                        out=pw[:],
                        lhsT=w2_sb[:, (k2 * JT + j) * 128:(k2 * JT + j + 1) * 128],
                        rhs=h_sb[:, k2 * BL:(k2 + 1) * BL],
                        start=(k2 == 0), stop=(k2 == HT - 1))
                nc.scalar.activation(out=negw32[:, j, :], in_=pw[:],
                                     func=AF.Identity,
                                     bias=nb2c_sb[:, j:j + 1], scale=-1.0)

            HB = BL // 2     # 16 batch cols per half-batch problem
            negw16 = [cpool.tile([128, JT * HB], F16, name=f"negw16_{p}")
                      for p in range(2)]
            for p in range(2):
                nc.vector.tensor_copy(
                    out=negw16[p][:],
                    in_=negw32[:, :, p * HB:(p + 1) * HB])

            # Two half-batch problems interleaved at block level: the PE runs
            # problem 1-p's 36 matmuls while the DVE MAX of problem p runs,
            # so the PE never waits at iteration boundaries.  Every psum
            # accumulation group (one j, k=0..5) is contiguous in issue
            # order -- interleaved groups within a PSUM bank lose partials.
            r = [negw16[0], negw16[1]]
            y_sb = cpool.tile([128, 2, JT * HB], F32)
            for it in range(k_iters):
                for p in range(2):
                    ps = ppool.tile([128, JT * HB], F32, tag=f"ps{p}",
                                    name=f"ps{it}_{p}")
                    for j in range(JT):
                        for k in range(JT):
                            nc.tensor.matmul(
                                out=ps[:, j * HB:(j + 1) * HB],
                                lhsT=G_sb[:, (k * JT + j) * 128:(k * JT + j + 1) * 128],
                                rhs=r[p][:, k * HB:(k + 1) * HB],
                                start=(k == 0), stop=(k == JT - 1))
                    if it < k_iters - 1:
                        tp = spool.tile([128, JT * HB], F16, tag=f"t{p}",
                                        name=f"t{it}_{p}")
                        nc.vector.tensor_tensor(out=tp[:], in0=ps[:],
                                                in1=negw16[p][:], op=AT.max)
                        r[p] = tp
                    else:
                        z = spool.tile([128, JT * HB], F32, tag=f"z{p}",
                                       name=f"z{p}")
                        nc.vector.scalar_tensor_tensor(
                            out=z[:], in0=ps[:], scalar=-1.0, in1=negw16[p][:],
                            op0=AT.mult, op1=AT.add)
                        nc.scalar.activation(out=y_sb[:, p, :], in_=z[:],
                                             func=AF.Relu, scale=1.0)
            nc.sync.dma_start(out=y_out[:], in_=y_sb[:])
    return nc


def _host_prepare(d, W1, b1, W2, b2, A, b_eq):
    A64 = A.astype(np.float64)
    M = np.linalg.pinv(A64 @ A64.T)
    G = A64.T @ M @ A64
    c = (b_eq.astype(np.float64) @ M) @ A64

    n2 = A.shape[1]
    NP = JT * 128
    G_pad = np.zeros((NP, NP), np.float64)
    G_pad[:n2, :n2] = G
    # fold c: pad row 767 carries -c; G[767,767]=1 keeps t[767]==1
    G_pad[NP - 1, :n2] = -c
    G_pad[NP - 1, NP - 1] = 1.0

    g_sb = (G_pad.reshape(JT, 128, JT, 128).transpose(1, 0, 2, 3)
            .reshape(128, JT * JT * 128)).astype(np.float16)

    HID = W1.shape[1]
    W2_pad = np.zeros((HID, NP), np.float64)
    W2_pad[:, :n2] = W2.astype(np.float64)
    w2_sb = (W2_pad.reshape(HT, 128, JT, 128).transpose(1, 0, 2, 3)
             .reshape(128, HT * JT * 128)).astype(np.float16)
    b1c = b1.reshape(HT, 128).T.astype(np.float32).copy()
    b2_pad = np.zeros(NP, np.float32)
    b2_pad[:n2] = b2
    b2_pad[NP - 1] = -1.0          # negw[767] = -b2_pad[767] = +1
    nb2c = (-b2_pad).reshape(JT, 128).T.astype(np.float32).copy()

    shared = {"g_mat": g_sb, "w2t": w2_sb, "w1": W1.astype(np.float16),
              "b1c": b1c, "nb2c": nb2c}
    B = d.shape[0]
    bl = B // N_CORES
    in_maps = []
    for i in range(N_CORES):
        dT = d[i * bl:(i + 1) * bl, :].T.astype(np.float16).copy()
        in_maps.append({**shared, "dt_in": dT})
    return in_maps


_nc_cache = {}


def kernel(d, W1, b1, W2, b2, A, b_eq):
    d = np.asarray(d, np.float32)
    W1 = np.asarray(W1, np.float32)
    b1 = np.asarray(b1, np.float32)
    W2 = np.asarray(W2, np.float32)
    b2 = np.asarray(b2, np.float32)
    A = np.asarray(A, np.float32)
    b_eq = np.asarray(b_eq, np.float32)

    if "nc" not in _nc_cache:
        _nc_cache["nc"] = _build()
    nc = _nc_cache["nc"]

    in_maps = _host_prepare(d, W1, b1, W2, b2, A, b_eq)
    res = run_bass_kernel_spmd(nc, in_maps, list(range(N_CORES)))

    outs = []
    for r in res.results:
        # y_out cols = prob*96 + j*16 + c;  batch = prob*16 + c, n2 = j*128 + p
        y = (r["y_out"].reshape(128, 2, JT, BL // 2).transpose(1, 3, 2, 0)
             .reshape(BL, JT * 128))
        outs.append(y[:, :N2])
    return np.concatenate(outs, axis=0).astype(np.float32)


# revision 15
# speedup vs baseline: 1.5149x; 1.5149x over previous
"""nn_Cvx_ShortestPathNet — TRN2 Bass kernel, 8-core pure data parallelism.

Math (derived from the reference's Dykstra iteration):
    G = A' pinv(AA') A   (orthogonal projector, 760x760), c = b' pinv(AA') A
    w = MLP(d); t_1 = -w
    for k = 1..K:  corr_k = t_k @ G - c ;  t_{k+1} = max(-w, corr_k)
    y = max(-w - corr_K, 0)
(The invariant t2 + p == -w collapses Dykstra's three-sequence state to a
single iterate.)

Schedule (v2):
  * c is folded into G: pad row 767 of G holds -c, G[767,767] = 1, and
    negw[767] is forced to 1, so t[767] == 1 is an invariant and the PE
    accumulates "t@G - c" directly in PSUM.  The per-iteration DVE work
    is then just 2 tensor_tensor MAX ops ([128,96] each) instead of 6
    scalar_tensor_tensor ops.
  * k-split pipelining: each iteration's 36 matmuls are emitted as
    phase A (contraction tiles k=0..2, reading tA) then phase B
    (k=3..5, reading tB).  The MAX producing the next tA only needs
    psum j=0..2, which completes 9 matmuls into phase B, so the DVE
    overlaps the PE and the PE never stalls at iteration boundaries.
  * all big operands (G, W1, W2, d) are fp16: halves the HBM traffic
    and keeps every matmul on the PE fast path.

Batch 256 is sharded 32 rows per core; G and MLP weights replicated.
"""

import json
import numpy as np

import concourse.bass as bass
import concourse.mybir as mybir
import concourse.tile as tile
from concourse.bass_utils import run_bass_kernel_spmd

F32 = mybir.dt.float32
F16 = mybir.dt.float16
AT = mybir.AluOpType
AF = mybir.ActivationFunctionType

JT = 6          # 768/128 edge-dim tiles
BL = 32         # batch rows per core
HT = 5          # 640/128 hidden tiles
K_ITERS = 100
N_CORES = 8
N2 = 760

# ---------------------------------------------------------------------------
# This container's walrus build rejects instructions carrying more than one
# sync-wait. Split any multi-wait instruction at the BIR-JSON level: insert
# same-engine NoOps before it, each carrying one of the extra waits (waits
# are sem-ge, so order is irrelevant).
_orig_to_json_bytes = bass.Bass.to_json_bytes
_ctr = [0]


def _split_waits_json(raw: bytes) -> bytes:
    j = json.loads(raw)
    changed = False
    for fn in j.get("functions", []):
        for bb in fn.get("blocks", []):
            out = []
            for inst in bb.get("instructions", []):
                si = inst.get("sync_info") or {}
                waits = si.get("on_wait") or []
                if len(waits) > 1:
                    changed = True
                    for w in waits[:-1]:
                        _ctr[0] += 1
                        out.append({
                            "debug": inst.get("debug", 0),
                            "engine": inst["engine"],
                            "ins": [], "outs": [],
                            "name": f"I-waitsplit-{_ctr[0]}",
                            "opcode": "NoOp",
                            "sync_info": {"on_wait": [w], "on_update": []},
                        })
                    si["on_wait"] = waits[-1:]
                out.append(inst)
            bb["instructions"] = out
    return json.dumps(j).encode() if changed else raw


def _patched_to_json_bytes(self, *a, **k):
    return _split_waits_json(_orig_to_json_bytes(self, *a, **k))


bass.Bass.to_json_bytes = _patched_to_json_bytes


def _build(k_iters=K_ITERS):
    nc = bass.Bass("TRN2", target_bir_lowering=False, debug=False,
                   num_devices=N_CORES)

    g_mat = nc.dram_tensor("g_mat", [128, JT * JT * 128], F16, kind="ExternalInput").ap()
    w2t = nc.dram_tensor("w2t", [128, HT * JT * 128], F16, kind="ExternalInput").ap()
    w1 = nc.dram_tensor("w1", [64, HT * 128], F16, kind="ExternalInput").ap()
    dt_in = nc.dram_tensor("dt_in", [64, BL], F16, kind="ExternalInput").ap()
    b1c = nc.dram_tensor("b1c", [128, HT], F32, kind="ExternalInput").ap()
    nb2c = nc.dram_tensor("nb2c", [128, JT], F32, kind="ExternalInput").ap()
    y_out = nc.dram_tensor("y_out", [128, JT * BL], F32, kind="ExternalOutput").ap()

    with tile.TileContext(nc) as tc:
        with (
            tc.tile_pool(name="const", bufs=1) as cpool,
            tc.tile_pool(name="state", bufs=2) as spool,
            tc.tile_pool(name="psum", bufs=2, space="PSUM") as ppool,
        ):
            # small loads first on the sync queue so the MLP starts early
            dT_sb = cpool.tile([64, BL], F16)
            nc.sync.dma_start(out=dT_sb[:], in_=dt_in[:])
            b1c_sb = cpool.tile([128, HT], F32)
            nc.sync.dma_start(out=b1c_sb[:], in_=b1c[:])
            nb2c_sb = cpool.tile([128, JT], F32)
            nc.sync.dma_start(out=nb2c_sb[:], in_=nb2c[:])
            w1_sb = cpool.tile([64, HT * 128], F16)
            nc.sync.dma_start(out=w1_sb[:], in_=w1[:])
            w2_sb = cpool.tile([128, HT * JT * 128], F16)
            nc.sync.dma_start(out=w2_sb[:], in_=w2t[:])
            # G on the SWDGE path so it overlaps the W2 load
            G_sb = cpool.tile([128, JT * JT * 128], F16)
            nc.gpsimd.dma_start(out=G_sb[:], in_=g_mat[:])

            # warm the scalar-engine activation table while DMAs run
            warm = spool.tile([128, 1], F32, tag="warm", name="warm")
            nc.scalar.activation(out=warm[:], in_=b1c_sb[:, 0:1],
                                 func=AF.Identity, scale=1.0)

            # MLP: h = leaky_relu(d@W1 + b1), negw = -(h@W2 + b2)
            h_sb = cpool.tile([128, HT * BL], F16)
            for m in range(HT):
                ph = ppool.tile([128, BL], F32, tag="mlp")
                nc.tensor.matmul(out=ph[:], lhsT=w1_sb[:, m * 128:(m + 1) * 128],
                                 rhs=dT_sb[:], start=True, stop=True)
                pre = spool.tile([128, BL], F32, tag="pre", name=f"pre{m}")
                nc.scalar.activation(out=pre[:], in_=ph[:], func=AF.Identity,
                                     bias=b1c_sb[:, m:m + 1], scale=1.0)
                # leaky relu: max(x, 0.1x) on DVE (Lrelu alpha is hardcoded
                # to 0.01 in this compiler build)
                nc.vector.scalar_tensor_tensor(
                    out=h_sb[:, m * BL:(m + 1) * BL], in0=pre[:],
                    scalar=0.1, in1=pre[:], op0=AT.mult, op1=AT.max)

            # negw: 6 activations into j-major [128, 6*32] fp16
            negw_sb = cpool.tile([128, JT * BL], F16)
            for j in range(JT):
                pw = ppool.tile([128, BL], F32, tag="mlp")
                for k2 in range(HT):
                    nc.tensor.matmul(
                        out=pw[:],
                        lhsT=w2_sb[:, (k2 * JT + j) * 128:(k2 * JT + j + 1) * 128],
                        rhs=h_sb[:, k2 * BL:(k2 + 1) * BL],
                        start=(k2 == 0), stop=(k2 == HT - 1))
                nc.scalar.activation(out=negw_sb[:, j * BL:(j + 1) * BL],
                                     in_=pw[:], func=AF.Identity,
                                     bias=nb2c_sb[:, j:j + 1], scale=-1.0)

            # One 32-wide problem.  Matmuls are LDWEIGHTS-bound (~26.6ns
            # each), so the floor is 36 weight loads per iteration; psum
            # accumulation groups must be contiguous in issue order
            # (interleaved groups in a PSUM bank lose partials).  The MAX
            # is split 3|2|1 j-tiles and emitted right after the group that
            # completes each psum tile, so only the last [128,32] MAX
            # remains on the critical path (~100ns/iter).
            # (source tile, column offset of k-tile 0 within it) per k-range
            rA, rB, rC = (negw_sb, 0), (negw_sb, 0), (negw_sb, 0)
            offB, offC = 3 * BL, 5 * BL
            y_sb = cpool.tile([128, JT * BL], F32)
            for it in range(k_iters):
                psA = ppool.tile([128, 3 * BL], F32, tag="psA", name=f"psA{it}")
                psB = ppool.tile([128, 2 * BL], F32, tag="psB", name=f"psB{it}")
                psC = ppool.tile([128, 1 * BL], F32, tag="psC", name=f"psC{it}")
                last = it == k_iters - 1
                for j in range(JT):
                    ps, sl = ((psA, slice(j * BL, (j + 1) * BL)) if j < 3 else
                              (psB, slice((j - 3) * BL, (j - 2) * BL)) if j < 5
                              else (psC, slice(0, BL)))
                    for k in range(JT):
                        src, off = rA if k < 3 else rB if k < 5 else rC
                        nc.tensor.matmul(
                            out=ps[:, sl],
                            lhsT=G_sb[:, (k * JT + j) * 128:(k * JT + j + 1) * 128],
                            rhs=src[:, k * BL - off:(k + 1) * BL - off],
                            start=(k == 0), stop=(k == JT - 1))
                    if j == 2 and not last:
                        tA = spool.tile([128, 3 * BL], F16, tag="tA",
                                        name=f"tA{it}")
                        with tc.high_priority():
                            nc.vector.tensor_tensor(out=tA[:], in0=psA[:],
                                                    in1=negw_sb[:, 0:3 * BL],
                                                    op=AT.max)
                    if j == 4 and not last:
                        tB = spool.tile([128, 2 * BL], F16, tag="tB",
                                        name=f"tB{it}")
                        with tc.high_priority():
                            nc.vector.tensor_tensor(out=tB[:], in0=psB[:],
                                                    in1=negw_sb[:, offB:5 * BL],
                                                    op=AT.max)
                if not last:
                    tC = spool.tile([128, 1 * BL], F16, tag="tC",
                                    name=f"tC{it}")
                    with tc.high_priority():
                        nc.vector.tensor_tensor(out=tC[:], in0=psC[:],
                                                in1=negw_sb[:, offC:6 * BL],
                                                op=AT.max)
                    rA, rB, rC = (tA, 0), (tB, offB), (tC, offC)
                else:
                    for ps, o, n in ((psA, 0, 3), (psB, offB, 2), (psC, offC, 1)):
                        z = spool.tile([128, n * BL], F32, tag=f"z{o}",
                                       name=f"z{o}")
                        nc.vector.scalar_tensor_tensor(
                            out=z[:], in0=ps[:], scalar=-1.0,
                            in1=negw_sb[:, o:o + n * BL],
                            op0=AT.mult, op1=AT.add)
                        nc.scalar.activation(out=y_sb[:, o:o + n * BL],
                                             in_=z[:], func=AF.Relu, scale=1.0)
            nc.sync.dma_start(out=y_out[:], in_=y_sb[:])
    return nc


def _host_prepare(d, W1, b1, W2, b2, A, b_eq):
    A64 = A.astype(np.float64)
    M = np.linalg.pinv(A64 @ A64.T)
    G = A64.T @ M @ A64
    c = (b_eq.astype(np.float64) @ M) @ A64

    n2 = A.shape[1]
    NP = JT * 128
    G_pad = np.zeros((NP, NP), np.float64)
    G_pad[:n2, :n2] = G
    # fold c: pad row 767 carries -c; G[767,767]=1 keeps t[767]==1
    G_pad[NP - 1, :n2] = -c
    G_pad[NP - 1, NP - 1] = 1.0

    g_sb = (G_pad.reshape(JT, 128, JT, 128).transpose(1, 0, 2, 3)
            .reshape(128, JT * JT * 128)).astype(np.float16)

    HID = W1.shape[1]
    W2_pad = np.zeros((HID, NP), np.float64)
    W2_pad[:, :n2] = W2.astype(np.float64)
    w2_sb = (W2_pad.reshape(HT, 128, JT, 128).transpose(1, 0, 2, 3)
             .reshape(128, HT * JT * 128)).astype(np.float16)
    b1c = b1.reshape(HT, 128).T.astype(np.float32).copy()
    b2_pad = np.zeros(NP, np.float32)
    b2_pad[:n2] = b2
    b2_pad[NP - 1] = -1.0          # negw[767] = -b2_pad[767] = +1
    nb2c = (-b2_pad).reshape(JT, 128).T.astype(np.float32).copy()

    shared = {"g_mat": g_sb, "w2t": w2_sb, "w1": W1.astype(np.float16),
              "b1c": b1c, "nb2c": nb2c}
    B = d.shape[0]
    bl = B // N_CORES
    in_maps = []
    for i in range(N_CORES):
        dT = d[i * bl:(i + 1) * bl, :].T.astype(np.float16).copy()
        in_maps.append({**shared, "dt_in": dT})
    return in_maps


_nc_cache = {}


def kernel(d, W1, b1, W2, b2, A, b_eq):
    d = np.asarray(d, np.float32)
    W1 = np.asarray(W1, np.float32)
    b1 = np.asarray(b1, np.float32)
    W2 = np.asarray(W2, np.float32)
    b2 = np.asarray(b2, np.float32)
    A = np.asarray(A, np.float32)
    b_eq = np.asarray(b_eq, np.float32)

    if "nc" not in _nc_cache:
        _nc_cache["nc"] = _build()
    nc = _nc_cache["nc"]

    in_maps = _host_prepare(d, W1, b1, W2, b2, A, b_eq)
    res = run_bass_kernel_spmd(nc, in_maps, list(range(N_CORES)))

    outs = []
    for r in res.results:
        y = (r["y_out"].reshape(128, JT, BL).transpose(2, 1, 0)
             .reshape(BL, JT * 128))
        outs.append(y[:, :N2])
    return np.concatenate(outs, axis=0).astype(np.float32)
